# revision 1
# baseline (speedup 1.0000x reference)
"""Trainium2 Bass kernel for nn_Awareness_5540507812461 (online kNN "Awareness" scan).

Algorithm recap (reference.py): a strictly sequential scan over B=4096 samples.
Step i computes distances from x_i to the current reference set, inserts x_i as
a new reference iff min-dist > R (R evolves from running min/max of distances),
and predicts the label of the nearest reference after insertion.

Key restructuring: if every step up to i inserted, the reference set at step i
is exactly {x_0..x_{i-1}}, so min_act_i / max_act_i are prefix min/max over row
i of the full pairwise-distance matrix — embarrassingly parallel.  The scalar
recurrence (min_d, max_d, R, insert) is then O(B) on 8 scalars.  The device
computes the full lower-triangle distance matrix (via TensorE matmuls) and the
per-row prefix min/max of d^2; the host runs the O(B) recurrence and VERIFIES
the all-insert speculation (decision margins on this workload are ~9.5 vs
distance scale ~45, far beyond fp32/bf16 noise).  When a step inserts, its
prediction is its own label (distance 0 to itself; min distance > 0 is also
verified).  If verification ever failed, an exact sequential fallback replays
the reference semantics on host.

Sharding: rows are interleaved across the 8 cores (core c owns rows
i == c mod 8), which makes the triangular work of every core IDENTICAL in
structure (one SPMD program, no per-core control flow) and balanced to within
one tile.  Each core owns 4 row-stripes of 128 rows; stripe k covers global
rows 1024k + 8p + c (p = partition index) and needs column blocks 0..2k+1.
Blocks 0..2k-1 are fully active (plain PSUM reduce); the two diagonal blocks
form one [128,1024] pair tile whose masked min uses a per-core staircase mask
(active iff q < 8p + c) applied with one tensor_tensor add of an SBUF +inf
mask, and whose max is taken over the raw superset (a superset max only raises
R, so the all-insert verification stays sound a-fortiori).

Distances: s[i,j] = ||xj||^2 - 2 xi.xj via one PSUM accumulation: 4 fp8-e4m3
DoubleRow matmuls (K=256 each; fp8 quantization error is bounded on the host
and folded into the verification slack) plus one bf16 matmul adding the norm
rows [n_hi_j; n_lo_j] against a ones stationary.  ||xi||^2 is a per-row
constant, added on host after the min/max reductions (does not affect
argmin/argmax ordering).
"""

import os
import sys

import numpy as np

B = 4096
D = 1024
NCORES = 8
NSTRIPE = 4  # row stripes per core, 128 rows each
NTILES = 20  # sum over stripes of (2k + 2)
F32INF = np.float32(np.inf)
WARMUP_MM = 10  # PE warmup matmuls issued while input DMAs stream in

_cached = {}


def _build_bass(reps=1):
    """Build (once per `reps`) the SPMD Bass program run on all 8 cores.
    reps>1 repeats the whole body (incl. input DMAs) for slope timing."""
    if ("nc", reps) in _cached:
        return _cached[("nc", reps)]
    sys.path.insert(0, "/opt/trn_rl_repo")
    import concourse.bass as bass
    import concourse.mybir as mybir
    from concourse.tile import TileContext

    nc = bass.Bass(trn_type="TRN2")
    f32 = mybir.dt.float32
    bf16 = mybir.dt.bfloat16

    f8 = mybir.dt.float8e4

    # rhs: x^T in fp8, split in 4 DoubleRow K-chunks of 256 (two interleaved
    # 128-row groups per chunk), halved into lo/hi column halves for earlier
    # compute start; the 2 norm rows ride separately in bf16.
    rhs_lo_d = nc.dram_tensor("rhs_lo", [4, 128, 2, B // 2], f8, kind="ExternalInput")
    rhs_hi_d = nc.dram_tensor("rhs_hi", [4, 128, 2, B // 2], f8, kind="ExternalInput")
    rhs8_d = nc.dram_tensor("rhs8", [2, B], bf16, kind="ExternalInput")
    lhs_d = nc.dram_tensor("lhsT", [4, 128, 2, 512], f8, kind="ExternalInput")
    mask_d = nc.dram_tensor("mask", [128, 1024], f32, kind="ExternalInput")
    mm_d = nc.dram_tensor("mm", [128, 2 * NSTRIPE], f32, kind="ExternalOutput")

    with TileContext(nc) as tc:
        with (
            tc.tile_pool(name="const", bufs=1) as cpool,
            tc.tile_pool(name="scratch", bufs=2) as spool,
            tc.tile_pool(name="psum", bufs=6, space="PSUM") as ppool,
            tc.tile_pool(name="ppsum", bufs=1, space="PSUM") as prpool,
        ):
            # ---- PE warmup while DMAs stream: matmuls on a memset dummy ----
            dummy = cpool.tile([128, 512], bf16, tag="dummy")
            nc.vector.memset(dummy[:], 0.0)
            ones2 = cpool.tile([2, 128], bf16, tag="ones2")
            nc.vector.memset(ones2[:], 1.0)
            wps = ppool.tile([128, 512], f32, tag="psum")
            for w in range(WARMUP_MM):
                nc.tensor.matmul(
                    wps[:], lhsT=dummy[:, 0:128], rhs=dummy[:],
                    start=(w == 0), stop=(w == WARMUP_MM - 1),
                )

            # ---- tiles shared across reps ----
            rhs8_t = cpool.tile([2, B], bf16, tag="rhs8")
            lhs_t = cpool.tile([128, 4, 2, 512], f8, tag="lhs")
            rhs_lo_t = cpool.tile([128, 4, 2, B // 2], f8, tag="rhslo")
            rhs_hi_t = cpool.tile([128, 4, 2, B // 2], f8, tag="rhshi")
            mask_t = cpool.tile([128, 1024], f32, tag="mask")
            acc_min = cpool.tile([128, 16], f32, tag="accmin")
            acc_max = cpool.tile([128, 16], f32, tag="accmax")
            res = cpool.tile([128, 2 * NSTRIPE], f32, tag="res")
            # fixed, contiguous per-stripe column ranges: stripe k owns
            # [base_k, base_k + 2k + 1) (2k fulls + 1 pair)
            acc_base = {0: 0, 1: 1, 2: 4, 3: 9}

            for _rep in range(reps):
              # ---- input DMAs, in consumption order ----
              # per-jb 512-column quarter DMAs, in tile consumption order
              # (Tile tracks region-level deps, so matmuls start as soon as
              # their own quarter has landed)
              for c4 in range(4):
                  nc.sync.dma_start(lhs_t[:, c4], lhs_d[c4])
              for c4 in range(4):
                  nc.sync.dma_start(
                      rhs_lo_t[:, c4, :, 0:512], rhs_lo_d[c4, :, :, 0:512])
              nc.sync.dma_start(rhs8_t[:], rhs8_d[:])
              nc.sync.dma_start(mask_t[:], mask_d[:])
              for q in range(1, 4):
                  for c4 in range(4):
                      nc.sync.dma_start(
                          rhs_lo_t[:, c4, :, q * 512 : (q + 1) * 512],
                          rhs_lo_d[c4, :, :, q * 512 : (q + 1) * 512])
              for q in range(4):
                  for c4 in range(4):
                      nc.sync.dma_start(
                          rhs_hi_t[:, c4, :, q * 512 : (q + 1) * 512],
                          rhs_hi_d[c4, :, :, q * 512 : (q + 1) * 512])

              acc_used = {k: 0 for k in range(NSTRIPE)}

              def _next_col(k):
                  t = acc_base[k] + acc_used[k]
                  acc_used[k] += 1
                  return t

              def mm_group(out_ap, k, jb0, ncols, close_group=True):
                  """Accumulate s = n_j - 2 x_i.x_j for columns [512*jb0,
                  512*(jb0+ncols)) of stripe k into PSUM out_ap."""
                  for j in range(ncols):
                      jb = jb0 + j
                      half = rhs_lo_t if jb < 4 else rhs_hi_t
                      q0 = (jb % 4) * 512
                      sl = out_ap[:, j * 512 : (j + 1) * 512]
                      for c4 in range(4):
                          nc.tensor.matmul(
                              sl,
                              lhsT=lhs_t[:, c4, :, k * 128 : (k + 1) * 128],
                              rhs=half[:, c4, :, q0 : q0 + 512],
                              perf_mode=mybir.MatmulPerfMode.DoubleRow,
                              start=(c4 == 0), stop=False,
                          )
                      nc.tensor.matmul(
                          sl, lhsT=ones2[:],
                          rhs=rhs8_t[:, jb * 512 : (jb + 1) * 512],
                          start=False, stop=close_group,
                          skip_group_check=not close_group,
                      )

              def full_tile(k, jb):
                  psum = ppool.tile([128, 512], f32, tag="psum")
                  mm_group(psum, k, jb, 1)
                  t = _next_col(k)
                  nc.vector.tensor_reduce(
                      acc_min[:, t : t + 1], psum[:],
                      axis=mybir.AxisListType.X, op=mybir.AluOpType.min,
                  )
                  nc.vector.tensor_reduce(
                      acc_max[:, t : t + 1], psum[:],
                      axis=mybir.AxisListType.X, op=mybir.AluOpType.max,
                  )

              def pair_tile(k):
                  # the two diagonal blocks jb=2k,2k+1 as one [128,1024] tile;
                  # min is masked (active q < 8p+c) via one tensor_tensor add
                  # of a +inf staircase; max is over the raw superset (only
                  # ever raises R -> verification stays sound)
                  psum = prpool.tile([128, 1024], f32, tag="ppsum")
                  mm_group(psum, k, 2 * k, 2)
                  t = _next_col(k)
                  masked = spool.tile([128, 1024], f32, tag="masked")
                  nc.vector.tensor_tensor(
                      out=masked[:], in0=psum[:], in1=mask_t[:],
                      op=mybir.AluOpType.add,
                  )
                  nc.vector.tensor_reduce(
                      acc_min[:, t : t + 1], masked[:],
                      axis=mybir.AxisListType.X, op=mybir.AluOpType.min,
                  )
                  nc.vector.tensor_reduce(
                      acc_max[:, t : t + 1], psum[:],
                      axis=mybir.AxisListType.X, op=mybir.AluOpType.max,
                  )

              # ---- main tiles: lo-column tiles first (hi DMAs still in
              # flight), DVE-heavy pairs early, cheap full tiles last ----
              full_tile(1, 0)         # jb 0 — smallest first-data footprint
              pair_tile(0)            # jb 0,1   (lo)
              pair_tile(1)            # jb 2,3   (lo)
              full_tile(1, 1)                             # lo
              full_tile(2, 0); full_tile(2, 1)            # lo
              full_tile(3, 0); full_tile(3, 1)            # lo
              full_tile(2, 2); full_tile(2, 3)            # lo
              pair_tile(2)            # jb 4,5   (hi)
              pair_tile(3)            # jb 6,7   (hi)
              full_tile(3, 2); full_tile(3, 3)            # lo
              full_tile(3, 4); full_tile(3, 5)            # hi

              # ---- per-stripe combine + output ----
              for k in range(NSTRIPE):
                  t0, n = acc_base[k], acc_used[k]
                  assert n == 2 * k + 1
                  nc.vector.tensor_reduce(
                      res[:, k : k + 1], acc_min[:, t0 : t0 + n],
                      axis=mybir.AxisListType.X, op=mybir.AluOpType.min,
                  )
                  nc.vector.tensor_reduce(
                      res[:, NSTRIPE + k : NSTRIPE + k + 1],
                      acc_max[:, t0 : t0 + n],
                      axis=mybir.AxisListType.X, op=mybir.AluOpType.max,
                  )
              nc.sync.dma_start(mm_d[:], res[:])

    _split_excess_waits(nc, mybir)
    _cached[("nc", reps)] = nc
    return nc


def _split_excess_waits(nc, mybir, ctrl_limit=1, other_limit=1):
    """This container's walrus build rejects >1 sync wait per instruction;
    hoist excess waits onto chained NoOps inserted before."""
    ctrl = {"Drain", "Nop", "NoOp"}
    n_split = 0
    for fn in nc.m.functions:
        for b in fn.blocks:
            insts = b.instructions
            i = 0
            while i < len(insts):
                ins = insts[i]
                limit = ctrl_limit if str(ins.opcode) in ctrl else other_limit
                si = getattr(ins, "sync_info", None)
                ow = list(si.on_wait) if si is not None and si.on_wait else []
                if len(ow) > limit:
                    si.on_wait = ow[:limit]
                    ins.sync_info = si
                    rest = ow[limit:]
                    pre = []
                    for j in range(0, len(rest), ctrl_limit):
                        n_split += 1
                        d = mybir.InstNoOp(name=f"I-wsplit-{n_split}")
                        d.engine = ins.engine
                        d.sync_info = mybir.SyncInfo(
                            on_wait=rest[j : j + ctrl_limit], on_update=[]
                        )
                        pre.append(d)
                    for j, d in enumerate(pre):
                        insts.insert(i + j, d)
                    i += len(pre)
                i += 1
    return n_split


def _prepare_inputs(xs):
    """Host-side shard/layout prep: fp8 cast, DoubleRow interleave, norm
    split, masks.  Returns (in_maps, n32, eps_max): n32 is ||x~_i||^2 fp32
    (added back on host), eps_max = max_i ||x~_i - x_i|| bounds the distance
    perturbation from fp8 quantization (|d~ - d| <= 2 eps_max)."""
    import ml_dtypes

    bf16 = ml_dtypes.bfloat16
    f8 = ml_dtypes.float8_e4m3
    x8 = xs.astype(f8)                        # [B, D] fp8 values
    xf = x8.astype(np.float32)
    eps_max = float(np.sqrt(((xf - xs) ** 2).sum(1)).max())
    n32 = np.einsum("ij,ij->i", xf.astype(np.float64), xf.astype(np.float64))
    n32 = n32.astype(np.float32)
    n_hi = n32.astype(bf16)
    n_lo = (n32 - n_hi.astype(np.float32)).astype(bf16)
    rhs8 = np.stack([n_hi, n_lo]).astype(bf16)  # [2, B]

    # DoubleRow layout: chunk c4 covers K rows [256 c4, 256 c4 + 256);
    # partition p carries rows (256 c4 + p, 256 c4 + 128 + p) in slots 0/1.
    xt = x8.T.reshape(4, 2, 128, B).transpose(0, 2, 1, 3)  # [4,128,2,B]
    rhs_lo = np.ascontiguousarray(xt[:, :, :, : B // 2])
    rhs_hi = np.ascontiguousarray(xt[:, :, :, B // 2 :])

    lhs_full = (xf.T * -2.0).astype(f8)       # [D, B]; -2*x exact in fp8
    lhs_dr = lhs_full.reshape(4, 2, 128, B).transpose(0, 2, 1, 3)

    p = np.arange(128)[:, None]
    tcol = np.arange(1024)[None, :]
    in_maps = []
    for c in range(NCORES):
        lhs_c = np.ascontiguousarray(lhs_dr[:, :, :, c::8])
        mask_c = np.where(tcol < 8 * p + c, np.float32(0.0), F32INF).astype(
            np.float32
        )
        in_maps.append(
            {"rhs_lo": rhs_lo, "rhs_hi": rhs_hi, "rhs8": rhs8,
             "lhsT": lhs_c, "mask": mask_c}
        )
    return in_maps, n32, eps_max


def _reassemble(results, n32):
    """Gather per-core [8,128] outputs into global m2/M2 [B] (d^2 min/max)."""
    m2 = np.empty(B, np.float32)
    M2 = np.empty(B, np.float32)
    for c, r in enumerate(results):
        mm = r["mm"]  # [128, 8]
        for k in range(NSTRIPE):
            rows = 1024 * k + 8 * np.arange(128) + c
            m2[rows] = mm[:, k]
            M2[rows] = mm[:, NSTRIPE + k]
    m2 = m2 + n32
    M2 = M2 + n32
    return m2, M2


def _scan_and_verify(m, M, slack):
    """Replay the reference's scalar recurrence under the all-insert
    speculation on the (perturbed) device distances; `slack` bounds the
    total perturbation effect (|m - m_true| <= slack_d, |R - R_true| <=
    2 slack_d / 3, folded into one threshold by the caller).  Returns
    (consistent-for-the-TRUE-dynamics, min margin)."""
    min_d = F32INF
    max_d = np.float32(0.0)
    R = np.float32(1.0)
    margin = np.inf
    for i in range(1, B):
        if not (np.isfinite(m[i]) and m[i] > slack):
            return False, -np.inf
        margin = min(margin, float(m[i] - R))
        if not (m[i] > R + slack):
            return False, margin
        min_d = np.float32(min(min_d, m[i]))
        max_d = np.float32(max(max_d, M[i]))
        R = np.float32((min_d + max_d) / np.float32(3.0))
    return True, margin


def _fallback_exact(xs, labels):
    """Exact sequential replay of the reference semantics (host, fp32)."""
    refs = np.zeros((B, D), np.float32)
    ref_labels = np.zeros((B,), np.float32)
    labels_f = labels.astype(np.float32)
    n_refs = 0
    min_d = F32INF
    max_d = np.float32(0.0)
    R = np.float32(1.0)
    preds = np.zeros(B, np.float32)
    for i in range(B):
        xi = xs[i]
        d_all = np.sqrt(np.sum((refs[:n_refs] - xi[None, :]) ** 2, axis=-1)).astype(
            np.float32
        )
        is_first = i == 0
        min_act = d_all.min() if n_refs else F32INF
        insert = is_first or (min_act > R)
        if insert:
            refs[n_refs] = xi
            ref_labels[n_refs] = labels_f[i]
        n2 = n_refs + int(insert)
        if not is_first:
            max_act = d_all.max() if n_refs else -F32INF
            min_d = np.float32(min(min_d, min_act))
            max_d = np.float32(max(max_d, max_act))
            R = np.float32((min_d + max_d) / np.float32(3.0))
        d2 = np.sqrt(np.sum((refs[:n2] - xi[None, :]) ** 2, axis=-1)).astype(np.float32)
        preds[i] = ref_labels[int(d2.argmin())]
        n_refs = n2
    return preds


def kernel(x, labels):
    x = np.asarray(x)
    labels = np.asarray(labels)
    xs = np.ascontiguousarray(x.reshape(B, D).astype(np.float32))

    sys.path.insert(0, "/opt/trn_rl_repo")
    from concourse.bass_utils import run_bass_kernel_spmd

    nc = _build_bass()
    in_maps, n32, eps_max = _prepare_inputs(xs)
    res = run_bass_kernel_spmd(nc, in_maps, core_ids=list(range(NCORES)))
    m2, M2 = _reassemble(res.results, n32)
    with np.errstate(invalid="ignore"):
        m = np.sqrt(np.maximum(m2, 0.0), dtype=np.float32)
        Mx = np.sqrt(np.maximum(M2, 0.0), dtype=np.float32)
        Mx = np.where(M2 < 0, np.float32(np.nan), Mx)
    # distance perturbation: fp8 quantization (2 eps_max) + norm bf16
    # rounding / fp32 accumulation (generous 0.1); R inherits 2/3 of it
    slack = np.float32((5.0 / 3.0) * (2.0 * eps_max + 0.1))
    ok, margin = _scan_and_verify(m, Mx, slack)
    if os.environ.get("AWARE_DEBUG"):
        print(f"[kernel] all-insert verified: {ok}, min margin: {margin:.4f}, "
              f"min dist: {np.nanmin(m[1:]):.4f}")
    if ok:
        # every step inserts and no zero-distance tie -> each sample predicts
        # its own label
        return labels.astype(np.float32)
    return _fallback_exact(xs, labels)


if __name__ == "__main__":
    rng = np.random.default_rng(0)
    x = rng.standard_normal((B, 1, D)).astype(np.float32)
    labels = rng.integers(0, 100, size=(B,)).astype(np.int64)
    out = kernel(x=x, labels=labels)
    print("kernel output:", out.shape, out.dtype, out[:8])



# revision 6
# speedup vs baseline: 1.4190x; 1.4190x over previous
"""Trainium2 Bass kernel for nn_Awareness_5540507812461 (online kNN "Awareness" scan).

Algorithm recap (reference.py): a strictly sequential scan over B=4096 samples.
Step i computes distances from x_i to the current reference set, inserts x_i as
a new reference iff min-dist > R (R evolves from running min/max of distances),
and predicts the label of the nearest reference after insertion.

Restructuring: if every step up to i inserted, the reference set at step i is
exactly {x_0..x_{i-1}}, so the per-step min/max distances are prefix extrema
over row i of the pairwise-distance matrix.  The device computes, per row i,
ell2[i] = min_{j<i} s_ij with s_ij = n_j - 2 x_i.x_j (so d^2 = n_i + s), plus
one GLOBAL max of s over all computed tiles (a superset of all j<i pairs; a
superset max only raises the evolving radius R, keeping verification sound).
The host adds n_i, replays the O(B) scalar recurrence with two-sided error
bounds, and verifies the all-insert speculation; margins on this workload are
~6 vs slack ~2.  On verification success each sample predicts its own label;
otherwise an exact sequential host fallback replays reference semantics.

Device scheme ("GM"):
- Distances via fp8(e4m3) DoubleRow matmuls only: the norm row n_j rides
  INSIDE the fp8 operands.  Dims 0..1019 carry x; rows 1020-1022 carry n_j
  split across 3 fp8 values at scales 32/2/1 (residual <= 0.0625); row 1023 is
  zero.  The lhsT carries -2x and the constants (32,2,1).  The dropped last-4
  data dims are bounded on host ((a4_i + prefix-max a4_j)^2, one-sided).
- Rows are interleaved across cores (core c owns rows i == c mod 8); each core
  has 4 row-stripes of 128; stripe k needs column big-tiles bt=0..k of 1024
  columns; bt==k is the diagonal "pair" tile, masked with a bf16 +-inf
  staircase via one tensor_tensor(max) (excluded elements -> +inf).
- PE: 40 DoubleRow matmuls of [K=256]x[1024 cols], c4-outer within 6 chunks of
  <=2 big-tiles so stationary weights reload only 24x; PSUM holds 2 chunks
  (2 x [128,2,1024] = all 8 banks); warmup matmuls run while inputs DMA in.
- ACT drains PSUM -> SBUF bf16 stage tiles (6 chunk-batched copies).
- DVE: staircase masks (tt-max vs +-inf), per-stripe tt-min trees, a chained
  tt-max global-max accumulator (ping-pong, no in-place ops), then halving
  tt-mins + one small reduce per output group (TensorReduce has no 2x mode,
  tensor_tensor does: all-bf16 operands run at 2 elem/cycle/lane).
"""

import os
import sys

import numpy as np

B = 4096
D = 1024
DDEV = 1020  # dims computed on device; last 4 carry the norm-split rows
NCORES = 8
NSTRIPE = 4
F32INF = np.float32(np.inf)
WARMUP_MM = 10
MASK_NEG = -3.0e38  # "pass" value for the staircase max-mask (active cols)

# chunks: (stripe k, [big-tile indices]); <=2 big-tiles each, c4-outer inside
CHUNKS = [
    (1, [0, 1]),
    (0, [0]),
    (2, [0, 1]),
    (3, [0, 1]),
    (2, [2]),
    (3, [2, 3]),
]
# stage slot offsets: stripe k's big-tile t lives at slot STAGE_OFF[k] + t
STAGE_OFF = {0: 0, 1: 1, 2: 3, 3: 6}

_cached = {}


def _build_bass(reps=1):
    """Build (once per `reps`) the SPMD Bass program run on all 8 cores."""
    if ("nc", reps) in _cached:
        return _cached[("nc", reps)]
    sys.path.insert(0, "/opt/trn_rl_repo")
    import concourse.bass as bass
    import concourse.mybir as mybir
    from concourse.tile import TileContext

    nc = bass.Bass(trn_type="TRN2")
    f32 = mybir.dt.float32
    bf16 = mybir.dt.bfloat16
    f8 = mybir.dt.float8e4

    rhs_d = nc.dram_tensor("rhs", [4, 128, 2, B], f8, kind="ExternalInput")
    lhs_d = nc.dram_tensor("lhsT", [4, 128, 2, 512], f8, kind="ExternalInput")
    mask_d = nc.dram_tensor("mask", [128, 1024], bf16, kind="ExternalInput")
    mm_d = nc.dram_tensor("mm", [128, 5], f32, kind="ExternalOutput")

    with TileContext(nc) as tc:
        with (
            tc.tile_pool(name="const", bufs=1) as cpool,
            tc.tile_pool(name="scratch", bufs=2) as spool,
            tc.tile_pool(name="psum", bufs=2, space="PSUM") as ppool,
        ):
            # ---- PE warmup while input DMAs stream ----
            dummy = cpool.tile([128, 512], bf16, tag="dummy")
            nc.vector.memset(dummy[:], 0.0)
            wps = ppool.tile([128, 2, 1024], f32, tag="psum")
            for w in range(WARMUP_MM):
                nc.tensor.matmul(
                    wps[:, 0, 0:512], lhsT=dummy[:, 0:128], rhs=dummy[:],
                    start=(w == 0), stop=(w == WARMUP_MM - 1),
                )

            # ---- persistent tiles ----
            rhs_t = cpool.tile([128, 4, 2, B], f8, tag="rhs")
            lhs_t = cpool.tile([128, 4, 2, 512], f8, tag="lhs")
            mask_t = cpool.tile([128, 1024], bf16, tag="mask")
            stage = cpool.tile([128, 10, 1024], bf16, tag="stage")
            maskout = cpool.tile([128, 4, 1024], bf16, tag="maskout")
            tree = cpool.tile([128, 2, 1024], bf16, tag="tree")
            roots = cpool.tile([128, 4, 1024], bf16, tag="roots")
            acc = cpool.tile([128, 2, 1024], bf16, tag="acc")
            res = cpool.tile([128, 8], f32, tag="res")

            for _rep in range(reps):
                # ---- input DMAs in consumption order ----
                for c4 in range(4):
                    nc.sync.dma_start(lhs_t[:, c4], lhs_d[c4])
                for c4 in range(4):
                    nc.sync.dma_start(
                        rhs_t[:, c4, :, 0:2048], rhs_d[c4, :, :, 0:2048])
                nc.sync.dma_start(mask_t[:], mask_d[:])
                for c4 in range(4):
                    nc.sync.dma_start(
                        rhs_t[:, c4, :, 2048:4096], rhs_d[c4, :, :, 2048:4096])

                # ---- per-chunk matmuls + ACT drain + DVE combine ----
                n_acc = [0]  # number of tiles folded into the gmax chain

                def gmax_fold(tile_ap):
                    """Chain tile into the global-max ping-pong accumulator."""
                    i = n_acc[0]
                    if i == 0:
                        # defer: first tile is just remembered via copy
                        nc.vector.tensor_tensor(
                            out=acc[:, 0, :], in0=tile_ap, in1=tile_ap,
                            op=mybir.AluOpType.max)
                    else:
                        nc.vector.tensor_tensor(
                            out=acc[:, i % 2, :], in0=acc[:, (i + 1) % 2, :],
                            in1=tile_ap, op=mybir.AluOpType.max)
                    n_acc[0] += 1

                stripe_tiles = {k: [] for k in range(NSTRIPE)}

                for (k, bts) in CHUNKS:
                    nbt = len(bts)
                    psum = ppool.tile([128, 2, 1024], f32, tag="psum")
                    for c4 in range(4):
                        for t, bt in enumerate(bts):
                            for h in range(2):
                                c0 = bt * 1024 + h * 512
                                nc.tensor.matmul(
                                    psum[:, t, h * 512:(h + 1) * 512],
                                    lhsT=lhs_t[:, c4, :, k * 128:(k + 1) * 128],
                                    rhs=rhs_t[:, c4, :, c0:c0 + 512],
                                    perf_mode=mybir.MatmulPerfMode.DoubleRow,
                                    start=(c4 == 0), stop=(c4 == 3),
                                    skip_group_check=True,
                                )
                    s0 = STAGE_OFF[k] + bts[0]
                    nc.scalar.copy(
                        stage[:, s0:s0 + nbt, :], psum[:, 0:nbt, :])
                    for bt in bts:
                        sl = STAGE_OFF[k] + bt
                        if bt == k:  # diagonal pair tile: staircase mask
                            out = (roots[:, 0, :] if k == 0
                                   else maskout[:, k, :])
                            nc.vector.tensor_tensor(
                                out=out, in0=stage[:, sl, :], in1=mask_t[:],
                                op=mybir.AluOpType.max)
                            if k != 0:
                                stripe_tiles[k].append(maskout[:, k, :])
                        else:
                            stripe_tiles[k].append(stage[:, sl, :])
                        gmax_fold(stage[:, sl, :])

                # ---- per-stripe min trees (stripe 0 root already done) ----
                t1 = stripe_tiles[1]
                nc.vector.tensor_tensor(
                    out=roots[:, 1, :], in0=t1[0], in1=t1[1],
                    op=mybir.AluOpType.min)
                t2 = stripe_tiles[2]
                nc.vector.tensor_tensor(
                    out=tree[:, 0, :], in0=t2[0], in1=t2[1],
                    op=mybir.AluOpType.min)
                nc.vector.tensor_tensor(
                    out=roots[:, 2, :], in0=tree[:, 0, :], in1=t2[2],
                    op=mybir.AluOpType.min)
                t3 = stripe_tiles[3]
                nc.vector.tensor_tensor(
                    out=tree[:, 1, :], in0=t3[0], in1=t3[1],
                    op=mybir.AluOpType.min)
                nc.vector.tensor_tensor(
                    out=tree[:, 0, :], in0=t3[2], in1=t3[3],
                    op=mybir.AluOpType.min)
                nc.vector.tensor_tensor(
                    out=roots[:, 3, :], in0=tree[:, 0, :], in1=tree[:, 1, :],
                    op=mybir.AluOpType.min)

                # ---- finals: halving tt-mins then small reduces ----
                rh = spool.tile([128, 4, 512], bf16, tag="rh")
                rq = spool.tile([128, 4, 256], bf16, tag="rq")
                nc.vector.tensor_tensor(
                    out=rh[:], in0=roots[:, :, 0:512], in1=roots[:, :, 512:1024],
                    op=mybir.AluOpType.min)
                nc.vector.tensor_tensor(
                    out=rq[:], in0=rh[:, :, 0:256], in1=rh[:, :, 256:512],
                    op=mybir.AluOpType.min)
                nc.vector.tensor_reduce(
                    res[:, 0:4], rq[:],
                    axis=mybir.AxisListType.X, op=mybir.AluOpType.min)

                afin = acc[:, (n_acc[0] + 1) % 2, :]  # last-written slice
                ah = spool.tile([128, 512], bf16, tag="ah")
                nc.vector.tensor_tensor(
                    out=ah[:], in0=afin[:, 0:512], in1=afin[:, 512:1024],
                    op=mybir.AluOpType.max)
                nc.vector.tensor_reduce(
                    res[:, 4:5], ah[:],
                    axis=mybir.AxisListType.X, op=mybir.AluOpType.max)

                nc.sync.dma_start(mm_d[:], res[:, 0:5])

    _split_excess_waits(nc, mybir)
    _cached[("nc", reps)] = nc
    return nc


def _split_excess_waits(nc, mybir, ctrl_limit=1, other_limit=1):
    """This container's walrus build rejects >1 sync wait per instruction;
    hoist excess waits onto chained NoOps inserted before."""
    ctrl = {"Drain", "Nop", "NoOp"}
    n_split = 0
    for fn in nc.m.functions:
        for b in fn.blocks:
            insts = b.instructions
            i = 0
            while i < len(insts):
                ins = insts[i]
                limit = ctrl_limit if str(ins.opcode) in ctrl else other_limit
                si = getattr(ins, "sync_info", None)
                ow = list(si.on_wait) if si is not None and si.on_wait else []
                if len(ow) > limit:
                    si.on_wait = ow[:limit]
                    ins.sync_info = si
                    rest = ow[limit:]
                    pre = []
                    for j in range(0, len(rest), ctrl_limit):
                        n_split += 1
                        d = mybir.InstNoOp(name=f"I-wsplit-{n_split}")
                        d.engine = ins.engine
                        d.sync_info = mybir.SyncInfo(
                            on_wait=rest[j : j + ctrl_limit], on_update=[]
                        )
                        pre.append(d)
                    for j, d in enumerate(pre):
                        insts.insert(i + j, d)
                    i += len(pre)
                i += 1
    return n_split


def _prepare_inputs(xs):
    """Host-side layout prep.  Returns (in_maps, host), where host carries the
    per-row quantities for the recurrence bounds:
      n1020: ||x~_i||^2 over dims 0..1019 (fp64->fp32)
      eps_max: max_i ||x~_i - x_i|| over all 1024 dims (fp8 quantization)
      r3_max: max norm-split residual (scales 32/2/1)
      a4: ||x~_i|| over dims 1020..1023 (dropped on device, bounded on host)
    """
    import ml_dtypes

    bf = ml_dtypes.bfloat16
    f8 = ml_dtypes.float8_e4m3
    x8 = xs.astype(f8)
    xf = x8.astype(np.float32)
    eps_max = float(np.sqrt(((xf - xs) ** 2).sum(1)).max())

    n1020 = np.einsum(
        "ij,ij->i", xf[:, :DDEV].astype(np.float64), xf[:, :DDEV].astype(np.float64)
    ).astype(np.float32)
    v1 = (n1020 / 32).astype(f8).astype(np.float32)
    r1 = n1020 - 32 * v1
    v2 = (r1 / 2).astype(f8).astype(np.float32)
    r2 = r1 - 2 * v2
    v3 = r2.astype(f8).astype(np.float32)
    r3_max = float(np.abs(r2 - v3).max())
    a4 = np.sqrt((xf[:, DDEV:] ** 2).sum(1)).astype(np.float32)

    # rhs: Xhat [1024, B] fp8: rows 0..1019 = x~^T, 1020-1022 = norm split, 1023 = 0
    Xhat = np.zeros((D, B), f8)
    Xhat[:DDEV] = x8.T[:DDEV]
    Xhat[DDEV + 0] = v1.astype(f8)
    Xhat[DDEV + 1] = v2.astype(f8)
    Xhat[DDEV + 2] = v3.astype(f8)
    rhs = np.ascontiguousarray(
        Xhat.reshape(4, 2, 128, B).transpose(0, 2, 1, 3))  # [4,128,2,B]

    # lhsT: XhatL [1024, B]: rows 0..1019 = -2 x~^T (exact), consts 32/2/1, 0
    XhatL = np.zeros((D, B), np.float32)
    XhatL[:DDEV] = xf.T[:DDEV] * -2.0
    XhatL[DDEV + 0] = 32.0
    XhatL[DDEV + 1] = 2.0
    XhatL[DDEV + 2] = 1.0
    XhatL8 = XhatL.astype(f8)
    lhs_dr = XhatL8.reshape(4, 2, 128, B).transpose(0, 2, 1, 3)

    p = np.arange(128)[:, None]
    q = np.arange(1024)[None, :]
    in_maps = []
    for c in range(NCORES):
        lhs_c = np.ascontiguousarray(lhs_dr[:, :, :, c::8])
        mask_c = np.where(q < 8 * p + c, np.float32(MASK_NEG),
                          np.float32(np.inf)).astype(bf)
        in_maps.append({"rhs": rhs, "lhsT": lhs_c, "mask": mask_c})
    host = {"n1020": n1020, "eps_max": eps_max, "r3_max": r3_max, "a4": a4}
    return in_maps, host


def _reassemble(results):
    """Gather per-core [128,8] outputs -> (ell2s [B] (min_j s, no n_i), gmax_s)."""
    mins = np.empty(B, np.float32)
    gmax_s = -np.inf
    for c, r in enumerate(results):
        mm = np.asarray(r["mm"], np.float32)  # [128, 8]
        for k in range(NSTRIPE):
            rows = 1024 * k + 8 * np.arange(128) + c
            mins[rows] = mm[:, k]
        gmax_s = max(gmax_s, float(mm[:, 4].max()))
    return mins, gmax_s


def _scan_and_verify(ell2, gmax_s, host):
    """Replay the scalar recurrence with sound two-sided bounds; return
    (all-insert verified for TRUE dynamics, min margin)."""
    n1020 = host["n1020"].astype(np.float64)
    eps2 = 2.0 * host["eps_max"]
    # s-scale slop: norm split + bf16 tile rounding (ulp<=8 at |s|<2048) + accum
    delta_s = host["r3_max"] + 8.0 + 0.05
    a4 = host["a4"].astype(np.float64)
    pm_a4 = np.maximum.accumulate(a4)
    d4max = np.zeros(B)
    d4max[1:] = (a4[1:] + pm_a4[:-1]) ** 2

    e2 = ell2.astype(np.float64) + n1020  # device ell^2 (+n_i), j<i min
    m_lo = np.sqrt(np.maximum(e2 - delta_s, 0.0)) - eps2
    m_hi = np.sqrt(np.maximum(e2 + delta_s + d4max, 0.0)) + eps2
    M_hi = np.sqrt(np.maximum(gmax_s + n1020 + delta_s + d4max, 0.0)) + eps2

    mind_hi = np.inf
    maxd_hi = 0.0
    R_hi = 1.0
    margin = np.inf
    for i in range(1, B):
        if not np.isfinite(m_lo[i]):
            return False, -np.inf
        margin = min(margin, float(m_lo[i] - R_hi))
        if not (m_lo[i] > R_hi and m_lo[i] > 0.0):
            return False, margin
        mind_hi = min(mind_hi, m_hi[i])
        maxd_hi = max(maxd_hi, M_hi[i])
        R_hi = (mind_hi + maxd_hi) / 3.0
    return True, margin


def _fallback_exact(xs, labels):
    """Exact sequential replay of the reference semantics (host, fp32)."""
    refs = np.zeros((B, D), np.float32)
    ref_labels = np.zeros((B,), np.float32)
    labels_f = labels.astype(np.float32)
    n_refs = 0
    min_d = F32INF
    max_d = np.float32(0.0)
    R = np.float32(1.0)
    preds = np.zeros(B, np.float32)
    for i in range(B):
        xi = xs[i]
        d_all = np.sqrt(np.sum((refs[:n_refs] - xi[None, :]) ** 2, axis=-1)).astype(
            np.float32
        )
        is_first = i == 0
        min_act = d_all.min() if n_refs else F32INF
        insert = is_first or (min_act > R)
        if insert:
            refs[n_refs] = xi
            ref_labels[n_refs] = labels_f[i]
        n2 = n_refs + int(insert)
        if not is_first:
            max_act = d_all.max() if n_refs else -F32INF
            min_d = np.float32(min(min_d, min_act))
            max_d = np.float32(max(max_d, max_act))
            R = np.float32((min_d + max_d) / np.float32(3.0))
        d2 = np.sqrt(np.sum((refs[:n2] - xi[None, :]) ** 2, axis=-1)).astype(np.float32)
        preds[i] = ref_labels[int(d2.argmin())]
        n_refs = n2
    return preds


def kernel(x, labels):
    x = np.asarray(x)
    labels = np.asarray(labels)
    xs = np.ascontiguousarray(x.reshape(B, D).astype(np.float32))

    sys.path.insert(0, "/opt/trn_rl_repo")
    from concourse.bass_utils import run_bass_kernel_spmd

    nc = _build_bass()
    in_maps, host = _prepare_inputs(xs)
    res = run_bass_kernel_spmd(nc, in_maps, core_ids=list(range(NCORES)))
    ell2, gmax_s = _reassemble(res.results)
    ok, margin = _scan_and_verify(ell2, gmax_s, host)
    if os.environ.get("AWARE_DEBUG"):
        e2 = ell2.astype(np.float64) + host["n1020"]
        with np.errstate(invalid="ignore"):
            md = np.sqrt(np.maximum(e2[1:], 0)).min()
        print(f"[kernel] all-insert verified: {ok}, min margin: {margin:.4f}, "
              f"min dist: {md:.4f}, gmax_s: {gmax_s:.1f}")
    if ok:
        return labels.astype(np.float32)
    return _fallback_exact(xs, labels)


if __name__ == "__main__":
    rng = np.random.default_rng(0)
    x = rng.standard_normal((B, 1, D)).astype(np.float32)
    labels = rng.integers(0, 100, size=(B,)).astype(np.int64)
    out = kernel(x=x, labels=labels)
    print("kernel output:", out.shape, out.dtype, out[:8])


# revision 32
# speedup vs baseline: 2.9080x; 2.0493x over previous
"""Trainium2 Bass kernel for nn_Awareness_5540507812461 (online kNN "Awareness" scan).

Algorithm recap (reference.py): a strictly sequential scan over B=4096 samples.
Step i computes distances from x_i to the current reference set, inserts x_i as
a new reference iff min-dist > R (R evolves from running min/max of distances),
and predicts the label of the nearest reference after insertion.

Restructuring: if every step up to i inserted, the reference set at step i is
exactly {x_0..x_{i-1}}, so the per-step min/max distances are prefix extrema
over row i of the pairwise-distance matrix.  The device computes, per row i,
ell2[i] = min_{j<i} s_ij with s_ij = n_j - 2 x_i.x_j (so d^2 = n_i + s), plus
one GLOBAL max of s over all computed tiles (a superset of all j<i pairs; a
superset max only raises the evolving radius R, keeping verification sound).
The host adds n_i, replays the O(B) scalar recurrence with two-sided error
bounds, and verifies the all-insert speculation; margins on this workload are
~6 vs slack ~2.  On verification success each sample predicts its own label;
otherwise an exact sequential host fallback replays reference semantics.

Device scheme ("GM"):
- Distances via fp8(e4m3) DoubleRow matmuls only: the norm row n_j rides
  INSIDE the fp8 operands.  Dims 0..1019 carry x; rows 1020-1022 carry n_j
  split across 3 fp8 values at scales 32/2/1 (residual <= 0.0625); row 1023 is
  zero.  The lhsT carries -2x and the constants (32,2,1).  The dropped last-4
  data dims are bounded on host ((a4_i + prefix-max a4_j)^2, one-sided).
- Rows are interleaved across cores (core c owns rows i == c mod 8); each core
  has 4 row-stripes of 128; stripe k needs column big-tiles bt=0..k of 1024
  columns; bt==k is the diagonal "pair" tile, masked with a bf16 +-inf
  staircase via one tensor_tensor(max) (excluded elements -> +inf).
- PE: 40 DoubleRow matmuls of [K=256]x[1024 cols], c4-outer within 6 chunks of
  <=2 big-tiles so stationary weights reload only 24x; PSUM holds 2 chunks
  (2 x [128,2,1024] = all 8 banks); warmup matmuls run while inputs DMA in.
- ACT drains PSUM -> SBUF bf16 stage tiles (6 chunk-batched copies).
- DVE: staircase masks (tt-max vs +-inf), per-stripe tt-min trees, a chained
  tt-max global-max accumulator (ping-pong, no in-place ops), then halving
  tt-mins + one small reduce per output group (TensorReduce has no 2x mode,
  tensor_tensor does: all-bf16 operands run at 2 elem/cycle/lane).
"""

import os
import sys

import numpy as np

B = 4096
D = 1024
DDEV = 1020  # dims computed on device; last 4 carry the norm-split rows
NCORES = 8
NSTRIPE = 4
F32INF = np.float32(np.inf)
WARMUP_MM = 6
MASK_NEG = -3.0e38  # "pass" value for the staircase max-mask (active cols)

# chunks: (stripe k, [big-tile indices]); <=2 big-tiles each, c4-outer inside.
# Ordered so the lo column half's readers (D,C,A,B) finish mid-rep -- the next
# rep's h0 DMAs then overlap this rep's h1 chunks (F,E) instead of stalling
# the next rep's matmuls.  The final chunk (E) is a single tile to keep the
# serial ACT+DVE tail short.
CHUNKS = [
    (3, [0, 1]),
    (2, [0, 1]),
    (1, [0, 1]),
    (0, [0]),
    (3, [2, 3]),
    (2, [2]),
]
# stage slot offsets: stripe k's big-tile t lives at slot STAGE_OFF[k] + t
STAGE_OFF = {0: 0, 1: 1, 2: 3, 3: 6}

_cached = {}


def _build_bass(reps=1):
    """Build (once per `reps`) the SPMD Bass program run on all 8 cores."""
    no_gmax = os.environ.get("AWARE_NO_GMAX") == "1"
    if ("nc", reps, no_gmax) in _cached:
        return _cached[("nc", reps, no_gmax)]
    sys.path.insert(0, "/opt/trn_rl_repo")
    import concourse.bass as bass
    import concourse.mybir as mybir
    from concourse.tile import TileContext

    nc = bass.Bass(trn_type="TRN2")
    f32 = mybir.dt.float32
    bf16 = mybir.dt.bfloat16
    f8 = mybir.dt.float8e4

    rhs_d = nc.dram_tensor("rhs", [128, 4, 2, B], f8, kind="ExternalInput")
    lhs_d = nc.dram_tensor("lhsT", [128, 4, 2, 512], f8, kind="ExternalInput")
    mask_d = nc.dram_tensor("mask", [128, 1024], bf16, kind="ExternalInput")
    mm_d = nc.dram_tensor("mm", [128, 6], f32, kind="ExternalOutput")

    with TileContext(nc) as tc:
        with (
            tc.tile_pool(name="const", bufs=1) as cpool,
            tc.tile_pool(name="scratch", bufs=2) as spool,
            tc.tile_pool(name="psum", bufs=4, space="PSUM") as ppool,
        ):
            # ---- PE warmup while input DMAs stream ----
            dummy = cpool.tile([128, 512], bf16, tag="dummy")
            nc.vector.memset(dummy[:], 0.0)
            wps = ppool.tile([128, 1024], f32, tag="psum")
            for w in range(WARMUP_MM):
                nc.tensor.matmul(
                    wps[:, 0:512], lhsT=dummy[:, 0:128], rhs=dummy[:],
                    start=(w == 0), stop=(w == WARMUP_MM - 1),
                )

            # ---- persistent tiles ----
            rhs_t = cpool.tile([128, 4, 2, B], f8, tag="rhs")
            mask_t = cpool.tile([128, 1024], bf16, tag="mask")
            stage = cpool.tile([128, 10, 1024], bf16, tag="stage")
            maskout = cpool.tile([128, 4, 1024], bf16, tag="maskout")
            tree = cpool.tile([128, 3, 1024], bf16, tag="tree")
            roots = cpool.tile([128, 4, 1024], bf16, tag="roots")
            acc = cpool.tile([128, 2, 1024], bf16, tag="acc")
            res = cpool.tile([128, 8], f32, tag="res")
            nc.vector.memset(res[:], 0.0)

            MIN = mybir.AluOpType.min
            MAX = mybir.AluOpType.max

            for _rep in range(reps):
                # ---- input DMAs in consumption order (lo half first) ----
                # lhs ping-pongs across reps (spool bufs=2) so the next rep's
                # lhs DMA never waits on this rep's readers
                lhs_t = spool.tile([128, 4, 2, 512], f8, tag="lhs")
                nc.sync.dma_start(lhs_t[:], lhs_d[:])
                for c4 in range(4):
                    nc.sync.dma_start(
                        rhs_t[:, c4, :, 0:2048], rhs_d[:, c4, :, 0:2048])
                nc.sync.dma_start(mask_t[:], mask_d[:])
                for c4 in range(4):
                    nc.sync.dma_start(
                        rhs_t[:, c4, :, 2048:4096], rhs_d[:, c4, :, 2048:4096])

                n_acc = [0]  # number of tiles folded into the gmax chain

                def gmax_fold(tile_ap, last=False):
                    """Chain tile into the global-max ping-pong accumulator.
                    The final tile (last=True) is instead max-reduced directly
                    into res[:,5] so the tail does not wait on the chain.
                    Skipped entirely in the no-gmax (host triangle bound)
                    variant."""
                    if no_gmax:
                        return
                    if last:
                        nc.vector.tensor_reduce(
                            res[:, 5:6], tile_ap,
                            axis=mybir.AxisListType.X, op=MAX)
                        return
                    i = n_acc[0]
                    if i == 0:
                        nc.vector.tensor_tensor(
                            out=acc[:, 0, :], in0=tile_ap, in1=tile_ap, op=MAX)
                    else:
                        nc.vector.tensor_tensor(
                            out=acc[:, i % 2, :], in0=acc[:, (i + 1) % 2, :],
                            in1=tile_ap, op=MAX)
                    n_acc[0] += 1

                def run_chunk(k, bts, last=False):
                    """MMs (c4-outer) + per-tile ACT drain + gmax links."""
                    psums = []
                    for _t in range(len(bts)):
                        psum = ppool.tile([128, 1024], f32, tag="psum")
                        psums.append(psum)
                    for c4 in range(4):
                        for t, bt in enumerate(bts):
                            for h in range(2):
                                c0 = bt * 1024 + h * 512
                                nc.tensor.matmul(
                                    psums[t][:, h * 512:(h + 1) * 512],
                                    lhsT=lhs_t[:, c4, :, k * 128:(k + 1) * 128],
                                    rhs=rhs_t[:, c4, :, c0:c0 + 512],
                                    perf_mode=mybir.MatmulPerfMode.DoubleRow,
                                    start=(c4 == 0), stop=(c4 == 3),
                                    skip_group_check=True,
                                )
                    for t, bt in enumerate(bts):
                        sl_ = STAGE_OFF[k] + bt
                        nc.scalar.copy(stage[:, sl_, :], psums[t][:])
                        gmax_fold(stage[:, sl_, :], last=(last and bt == bts[-1]))

                def sl(k, bt):
                    return stage[:, STAGE_OFF[k] + bt, :]

                def finals(k):
                    nc.vector.tensor_reduce(
                        res[:, k:k + 1], roots[:, k, :],
                        axis=mybir.AxisListType.X, op=MIN)

                # chunk (3,[0,1]): treeA
                run_chunk(3, [0, 1])
                nc.vector.tensor_tensor(
                    out=tree[:, 0, :], in0=sl(3, 0), in1=sl(3, 1), op=MIN)
                # chunk (2,[0,1]): treeC
                run_chunk(2, [0, 1])
                nc.vector.tensor_tensor(
                    out=tree[:, 2, :], in0=sl(2, 0), in1=sl(2, 1), op=MIN)
                # chunk (1,[0,1]): mask bt1, root1, finals(1)
                run_chunk(1, [0, 1])
                nc.vector.tensor_tensor(
                    out=maskout[:, 1, :], in0=sl(1, 1), in1=mask_t[:], op=MAX)
                nc.vector.tensor_tensor(
                    out=roots[:, 1, :], in0=sl(1, 0), in1=maskout[:, 1, :],
                    op=MIN)
                finals(1)
                # chunk (0,[0]): mask -> root0, finals(0)
                run_chunk(0, [0])
                nc.vector.tensor_tensor(
                    out=roots[:, 0, :], in0=sl(0, 0), in1=mask_t[:], op=MAX)
                finals(0)
                # chunk (3,[2,3]): mask bt3, treeB, root3, finals(3)
                run_chunk(3, [2, 3])
                nc.vector.tensor_tensor(
                    out=maskout[:, 3, :], in0=sl(3, 3), in1=mask_t[:], op=MAX)
                nc.vector.tensor_tensor(
                    out=tree[:, 1, :], in0=sl(3, 2), in1=maskout[:, 3, :],
                    op=MIN)
                nc.vector.tensor_tensor(
                    out=roots[:, 3, :], in0=tree[:, 0, :], in1=tree[:, 1, :],
                    op=MIN)
                finals(3)
                # reduce the 9-tile gmax chain (off the tail)
                if not no_gmax:
                    afin = acc[:, (n_acc[0] + 1) % 2, :]  # last-written slice
                    nc.vector.tensor_reduce(
                        res[:, 4:5], afin,
                        axis=mybir.AxisListType.X, op=MAX)
                # chunk (2,[2]) last: tile 10 max-reduced directly (last=True);
                # mask, root2, finals(2)
                run_chunk(2, [2], last=True)
                nc.vector.tensor_tensor(
                    out=maskout[:, 2, :], in0=sl(2, 2), in1=mask_t[:], op=MAX)
                nc.vector.tensor_tensor(
                    out=roots[:, 2, :], in0=tree[:, 2, :], in1=maskout[:, 2, :],
                    op=MIN)
                finals(2)

                nc.sync.dma_start(mm_d[:], res[:, 0:6])

    if os.environ.get("AWARE_NO_LDW_DEDUP") != "1":
        _dedup_ldweights(nc, mybir)
    _split_excess_waits(nc, mybir)
    _cached[("nc", reps, no_gmax)] = nc
    return nc


def _dedup_ldweights(nc, mybir):
    """Drop Ldweights whose stationary operand is identical to the previous
    Ldweights on the PE stream (weights persist in the array between matmuls).
    Only drops instructions with no sync waits; any on_update is migrated to
    the previous PE instruction."""
    def key(ins):
        ap = ins.ins[0]
        return (getattr(ap, "memref", None), getattr(ap, "offset", None),
                str(getattr(ap, "ap", None)), str(getattr(ap, "dtype", None)),
                str(getattr(ins, "perf_mode", None)),
                str(getattr(ins, "is_transpose", None)))

    n_drop = 0
    for fn in nc.m.functions:
        for b in fn.blocks:
            insts = b.instructions
            prev_key = [None]
            keep = []
            for ins in insts:
                op = str(ins.opcode)
                if getattr(ins, "engine", None) != mybir.EngineType.PE:
                    keep.append(ins)
                    continue
                if op == "Ldweights":
                    si = getattr(ins, "sync_info", None)
                    waits = list(si.on_wait) if si is not None and si.on_wait else []
                    upds = list(si.on_update) if si is not None and si.on_update else []
                    if key(ins) == prev_key[0] and not waits and not upds:
                        n_drop += 1
                        continue
                    prev_key[0] = key(ins)
                    keep.append(ins)
                else:
                    if op not in ("Matmult",):
                        prev_key[0] = None  # unknown PE op may clobber array
                    keep.append(ins)
            b.instructions = keep
    return n_drop


def _split_excess_waits(nc, mybir, ctrl_limit=1, other_limit=1):
    """This container's walrus build rejects >1 sync wait per instruction;
    hoist excess waits onto chained NoOps inserted before."""
    ctrl = {"Drain", "Nop", "NoOp"}
    n_split = 0
    for fn in nc.m.functions:
        for b in fn.blocks:
            insts = b.instructions
            i = 0
            while i < len(insts):
                ins = insts[i]
                limit = ctrl_limit if str(ins.opcode) in ctrl else other_limit
                si = getattr(ins, "sync_info", None)
                ow = list(si.on_wait) if si is not None and si.on_wait else []
                if len(ow) > limit:
                    si.on_wait = ow[:limit]
                    ins.sync_info = si
                    rest = ow[limit:]
                    pre = []
                    for j in range(0, len(rest), ctrl_limit):
                        n_split += 1
                        d = mybir.InstNoOp(name=f"I-wsplit-{n_split}")
                        d.engine = ins.engine
                        d.sync_info = mybir.SyncInfo(
                            on_wait=rest[j : j + ctrl_limit], on_update=[]
                        )
                        pre.append(d)
                    for j, d in enumerate(pre):
                        insts.insert(i + j, d)
                    i += len(pre)
                i += 1
    return n_split


def _prepare_inputs(xs):
    """Host-side layout prep.  Returns (in_maps, host), where host carries the
    per-row quantities for the recurrence bounds:
      n1020: ||x~_i||^2 over dims 0..1019 (fp64->fp32)
      eps_max: max_i ||x~_i - x_i|| over all 1024 dims (fp8 quantization)
      r3_max: max norm-split residual (scales 32/2/1)
      a4: ||x~_i|| over dims 1020..1023 (dropped on device, bounded on host)
    """
    import ml_dtypes

    bf = ml_dtypes.bfloat16
    f8 = ml_dtypes.float8_e4m3
    x8 = xs.astype(f8)
    xf = x8.astype(np.float32)
    eps_max = float(np.sqrt(((xf - xs) ** 2).sum(1)).max())

    n1020 = np.einsum(
        "ij,ij->i", xf[:, :DDEV].astype(np.float64), xf[:, :DDEV].astype(np.float64)
    ).astype(np.float32)
    v1 = (n1020 / 32).astype(f8).astype(np.float32)
    r1 = n1020 - 32 * v1
    v2 = (r1 / 2).astype(f8).astype(np.float32)
    r2 = r1 - 2 * v2
    v3 = r2.astype(f8).astype(np.float32)
    r3_max = float(np.abs(r2 - v3).max())
    a4 = np.sqrt((xf[:, DDEV:] ** 2).sum(1)).astype(np.float32)
    bnorm = np.sqrt((xs.astype(np.float64) ** 2).sum(1))  # true ||x_i||

    # rhs: Xhat [1024, B] fp8: rows 0..1019 = x~^T, 1020-1022 = norm split, 1023 = 0
    Xhat = np.zeros((D, B), f8)
    Xhat[:DDEV] = x8.T[:DDEV]
    Xhat[DDEV + 0] = v1.astype(f8)
    Xhat[DDEV + 1] = v2.astype(f8)
    Xhat[DDEV + 2] = v3.astype(f8)
    rhs = np.ascontiguousarray(
        Xhat.reshape(4, 2, 128, B).transpose(2, 0, 1, 3))  # [128,4,2,B]

    # lhsT: XhatL [1024, B]: rows 0..1019 = -2 x~^T (exact), consts 32/2/1, 0
    XhatL = np.zeros((D, B), np.float32)
    XhatL[:DDEV] = xf.T[:DDEV] * -2.0
    XhatL[DDEV + 0] = 32.0
    XhatL[DDEV + 1] = 2.0
    XhatL[DDEV + 2] = 1.0
    XhatL8 = XhatL.astype(f8)
    lhs_dr = XhatL8.reshape(4, 2, 128, B).transpose(2, 0, 1, 3)  # [128,4,2,B]

    p = np.arange(128)[:, None]
    q = np.arange(1024)[None, :]
    in_maps = []
    for c in range(NCORES):
        lhs_c = np.ascontiguousarray(lhs_dr[:, :, :, c::8])
        mask_c = np.where(q < 8 * p + c, np.float32(MASK_NEG),
                          np.float32(np.inf)).astype(bf)
        in_maps.append({"rhs": rhs, "lhsT": lhs_c, "mask": mask_c})
    host = {"n1020": n1020, "eps_max": eps_max, "r3_max": r3_max, "a4": a4,
            "bnorm": bnorm}
    return in_maps, host


def _reassemble(results):
    """Gather per-core [128,6] outputs -> (ell2s [B] (min_j s, no n_i), gmax_s)."""
    mins = np.empty(B, np.float32)
    gmax_s = -np.inf
    for c, r in enumerate(results):
        mm = np.asarray(r["mm"], np.float32)  # [128, 6]
        for k in range(NSTRIPE):
            rows = 1024 * k + 8 * np.arange(128) + c
            mins[rows] = mm[:, k]
        gmax_s = max(gmax_s, float(mm[:, 4:6].max()))
    return mins, gmax_s


def _scan_and_verify(ell2, gmax_s, host):
    """Replay the scalar recurrence with sound two-sided bounds; return
    (all-insert verified for TRUE dynamics, min margin)."""
    n1020 = host["n1020"].astype(np.float64)
    eps2 = 2.0 * host["eps_max"]
    # s-scale slop: norm split + bf16 tile rounding (ulp<=8 at |s|<2048) + accum
    delta_s = host["r3_max"] + 8.0 + 0.05
    a4 = host["a4"].astype(np.float64)
    pm_a4 = np.maximum.accumulate(a4)
    d4max = np.zeros(B)
    d4max[1:] = (a4[1:] + pm_a4[:-1]) ** 2

    e2 = ell2.astype(np.float64) + n1020  # device ell^2 (+n_i), j<i min
    m_lo = np.sqrt(np.maximum(e2 - delta_s, 0.0)) - eps2
    m_hi = np.sqrt(np.maximum(e2 + delta_s + d4max, 0.0)) + eps2
    if np.isfinite(gmax_s):
        M_hi = np.sqrt(np.maximum(gmax_s + n1020 + delta_s + d4max, 0.0)) + eps2
    else:
        # no-gmax variant: triangle-inequality bound on the true (unquantized)
        # distances, max_{j<i} d_ij <= ||x_i|| + max_{j<i} ||x_j||
        b = host["bnorm"].astype(np.float64)
        pm_b = np.maximum.accumulate(b)
        M_hi = np.empty(B)
        M_hi[0] = 0.0
        M_hi[1:] = b[1:] + pm_b[:-1]

    mind_hi = np.inf
    maxd_hi = 0.0
    R_hi = 1.0
    margin = np.inf
    for i in range(1, B):
        if not np.isfinite(m_lo[i]):
            return False, -np.inf
        margin = min(margin, float(m_lo[i] - R_hi))
        if not (m_lo[i] > R_hi and m_lo[i] > 0.0):
            return False, margin
        mind_hi = min(mind_hi, m_hi[i])
        maxd_hi = max(maxd_hi, M_hi[i])
        R_hi = (mind_hi + maxd_hi) / 3.0
    return True, margin


def _fallback_exact(xs, labels):
    """Exact sequential replay of the reference semantics (host, fp32)."""
    refs = np.zeros((B, D), np.float32)
    ref_labels = np.zeros((B,), np.float32)
    labels_f = labels.astype(np.float32)
    n_refs = 0
    min_d = F32INF
    max_d = np.float32(0.0)
    R = np.float32(1.0)
    preds = np.zeros(B, np.float32)
    for i in range(B):
        xi = xs[i]
        d_all = np.sqrt(np.sum((refs[:n_refs] - xi[None, :]) ** 2, axis=-1)).astype(
            np.float32
        )
        is_first = i == 0
        min_act = d_all.min() if n_refs else F32INF
        insert = is_first or (min_act > R)
        if insert:
            refs[n_refs] = xi
            ref_labels[n_refs] = labels_f[i]
        n2 = n_refs + int(insert)
        if not is_first:
            max_act = d_all.max() if n_refs else -F32INF
            min_d = np.float32(min(min_d, min_act))
            max_d = np.float32(max(max_d, max_act))
            R = np.float32((min_d + max_d) / np.float32(3.0))
        d2 = np.sqrt(np.sum((refs[:n2] - xi[None, :]) ** 2, axis=-1)).astype(np.float32)
        preds[i] = ref_labels[int(d2.argmin())]
        n_refs = n2
    return preds


def kernel(x, labels):
    x = np.asarray(x)
    labels = np.asarray(labels)
    xs = np.ascontiguousarray(x.reshape(B, D).astype(np.float32))

    sys.path.insert(0, "/opt/trn_rl_repo")
    from concourse.bass_utils import run_bass_kernel_spmd

    nc = _build_bass()
    in_maps, host = _prepare_inputs(xs)
    res = run_bass_kernel_spmd(nc, in_maps, core_ids=list(range(NCORES)))
    ell2, gmax_s = _reassemble(res.results)
    if os.environ.get("AWARE_NO_GMAX") == "1":
        gmax_s = np.nan  # host triangle bound instead
    ok, margin = _scan_and_verify(ell2, gmax_s, host)
    if os.environ.get("AWARE_DEBUG"):
        e2 = ell2.astype(np.float64) + host["n1020"]
        with np.errstate(invalid="ignore"):
            md = np.sqrt(np.maximum(e2[1:], 0)).min()
        print(f"[kernel] all-insert verified: {ok}, min margin: {margin:.4f}, "
              f"min dist: {md:.4f}, gmax_s: {gmax_s:.1f}")
    if ok:
        return labels.astype(np.float32)
    return _fallback_exact(xs, labels)


if __name__ == "__main__":
    rng = np.random.default_rng(0)
    x = rng.standard_normal((B, 1, D)).astype(np.float32)
    labels = rng.integers(0, 100, size=(B,)).astype(np.int64)
    out = kernel(x=x, labels=labels)
    print("kernel output:", out.shape, out.dtype, out[:8])


# revision 33
# speedup vs baseline: 2.9700x; 1.0213x over previous
"""Trainium2 Bass kernel for nn_Awareness_5540507812461 (online kNN "Awareness" scan).

Algorithm recap (reference.py): a strictly sequential scan over B=4096 samples.
Step i computes distances from x_i to the current reference set, inserts x_i as
a new reference iff min-dist > R (R evolves from running min/max of distances),
and predicts the label of the nearest reference after insertion.

Restructuring: if every step up to i inserted, the reference set at step i is
exactly {x_0..x_{i-1}}, so the per-step min/max distances are prefix extrema
over row i of the pairwise-distance matrix.  The device computes, per row i,
ell2[i] = min_{j<i} s_ij with s_ij = n_j - 2 x_i.x_j (so d^2 = n_i + s), plus
one GLOBAL max of s over all computed tiles (a superset of all j<i pairs; a
superset max only raises the evolving radius R, keeping verification sound).
The host adds n_i, replays the O(B) scalar recurrence with two-sided error
bounds, and verifies the all-insert speculation; margins on this workload are
~6 vs slack ~2.  On verification success each sample predicts its own label;
otherwise an exact sequential host fallback replays reference semantics.

Device scheme ("GM"):
- Distances via fp8(e4m3) DoubleRow matmuls only: the norm row n_j rides
  INSIDE the fp8 operands.  Dims 0..1019 carry x; rows 1020-1022 carry n_j
  split across 3 fp8 values at scales 32/2/1 (residual <= 0.0625); row 1023 is
  zero.  The lhsT carries -2x and the constants (32,2,1).  The dropped last-4
  data dims are bounded on host ((a4_i + prefix-max a4_j)^2, one-sided).
- Rows are interleaved across cores (core c owns rows i == c mod 8); each core
  has 4 row-stripes of 128; stripe k needs column big-tiles bt=0..k of 1024
  columns; bt==k is the diagonal "pair" tile, masked with a bf16 +-inf
  staircase via one tensor_tensor(max) (excluded elements -> +inf).
- PE: 40 DoubleRow matmuls of [K=256]x[1024 cols], c4-outer within 6 chunks of
  <=2 big-tiles so stationary weights reload only 24x; PSUM holds 2 chunks
  (2 x [128,2,1024] = all 8 banks); warmup matmuls run while inputs DMA in.
- ACT drains PSUM -> SBUF bf16 stage tiles (6 chunk-batched copies).
- DVE: staircase masks (tt-max vs +-inf), per-stripe tt-min trees, a chained
  tt-max global-max accumulator (ping-pong, no in-place ops), then halving
  tt-mins + one small reduce per output group (TensorReduce has no 2x mode,
  tensor_tensor does: all-bf16 operands run at 2 elem/cycle/lane).
"""

import os
import sys

import numpy as np

B = 4096
D = 1024
DDEV = 1020  # dims computed on device; last 4 carry the norm-split rows
NCORES = 8
NSTRIPE = 4
F32INF = np.float32(np.inf)
WARMUP_MM = 6
MASK_NEG = -3.0e38  # "pass" value for the staircase max-mask (active cols)

# chunks: (stripe k, [big-tile indices]); <=2 big-tiles each, c4-outer inside.
# Ordered so the lo column half's readers (D,C,A,B) finish mid-rep -- the next
# rep's h0 DMAs then overlap this rep's h1 chunks (F,E) instead of stalling
# the next rep's matmuls.  The final chunk (E) is a single tile to keep the
# serial ACT+DVE tail short.
CHUNKS = [
    (3, [0, 1]),
    (2, [0, 1]),
    (1, [0, 1]),
    (0, [0]),
    (3, [2, 3]),
    (2, [2]),
]
# stage slot offsets: stripe k's big-tile t lives at slot STAGE_OFF[k] + t
STAGE_OFF = {0: 0, 1: 1, 2: 3, 3: 6}

_cached = {}


def _build_bass(reps=1):
    """Build (once per `reps`) the SPMD Bass program run on all 8 cores."""
    no_gmax = os.environ.get("AWARE_NO_GMAX") == "1"
    if ("nc", reps, no_gmax) in _cached:
        return _cached[("nc", reps, no_gmax)]
    sys.path.insert(0, "/opt/trn_rl_repo")
    import concourse.bass as bass
    import concourse.mybir as mybir
    from concourse.tile import TileContext

    nc = bass.Bass(trn_type="TRN2")
    f32 = mybir.dt.float32
    bf16 = mybir.dt.bfloat16
    f8 = mybir.dt.float8e4

    rhs_d = nc.dram_tensor("rhs", [128, 4, 2, B], f8, kind="ExternalInput")
    lhs_d = nc.dram_tensor("lhsT", [128, 4, 2, 512], f8, kind="ExternalInput")
    mask_d = nc.dram_tensor("mask", [128, 1024], bf16, kind="ExternalInput")
    mm_d = nc.dram_tensor("mm", [128, 6], f32, kind="ExternalOutput")

    with TileContext(nc) as tc:
        with (
            tc.tile_pool(name="const", bufs=1) as cpool,
            tc.tile_pool(name="scratch", bufs=2) as spool,
            tc.tile_pool(name="psum", bufs=4, space="PSUM") as ppool,
        ):
            # ---- PE warmup while input DMAs stream ----
            dummy = cpool.tile([128, 512], bf16, tag="dummy")
            nc.vector.memset(dummy[:], 0.0)
            wps = ppool.tile([128, 1024], f32, tag="psum")
            for w in range(WARMUP_MM):
                nc.tensor.matmul(
                    wps[:, 0:512], lhsT=dummy[:, 0:128], rhs=dummy[:],
                    start=(w == 0), stop=(w == WARMUP_MM - 1),
                )

            # ---- persistent tiles ----
            rhs_t = cpool.tile([128, 4, 2, B], f8, tag="rhs")
            mask_t = cpool.tile([128, 1024], bf16, tag="mask")
            stage = cpool.tile([128, 10, 1024], bf16, tag="stage")
            maskout = cpool.tile([128, 4, 1024], bf16, tag="maskout")
            tree = cpool.tile([128, 3, 1024], bf16, tag="tree")
            roots = cpool.tile([128, 4, 1024], bf16, tag="roots")
            acc = cpool.tile([128, 2, 1024], bf16, tag="acc")
            res = cpool.tile([128, 8], f32, tag="res")
            nc.vector.memset(res[:], 0.0)

            MIN = mybir.AluOpType.min
            MAX = mybir.AluOpType.max

            for _rep in range(reps):
                # ---- input DMAs in consumption order (lo half first) ----
                # lhs ping-pongs across reps (spool bufs=2) so the next rep's
                # lhs DMA never waits on this rep's readers
                lhs_t = spool.tile([128, 4, 2, 512], f8, tag="lhs")
                nc.sync.dma_start(lhs_t[:], lhs_d[:])
                for c4 in range(4):
                    nc.sync.dma_start(
                        rhs_t[:, c4, :, 0:2048], rhs_d[:, c4, :, 0:2048])
                nc.sync.dma_start(mask_t[:], mask_d[:])
                for c4 in range(4):
                    nc.sync.dma_start(
                        rhs_t[:, c4, :, 2048:4096], rhs_d[:, c4, :, 2048:4096])

                n_acc = [0]  # number of links in the gmax chain
                pend = [None]  # first tile waits to pair with the second

                def gmax_fold(tile_ap, last=False):
                    """Chain tile into the global-max ping-pong accumulator.
                    The final tile (last=True) is instead max-reduced directly
                    into res[:,5] so the tail does not wait on the chain.
                    Skipped entirely in the no-gmax (host triangle bound)
                    variant."""
                    if no_gmax:
                        return
                    if last:
                        nc.vector.tensor_reduce(
                            res[:, 5:6], tile_ap,
                            axis=mybir.AxisListType.X, op=MAX)
                        return
                    if n_acc[0] == 0 and pend[0] is None:
                        pend[0] = tile_ap
                        return
                    i = n_acc[0]
                    src = acc[:, (i + 1) % 2, :] if pend[0] is None else pend[0]
                    pend[0] = None
                    nc.vector.tensor_tensor(
                        out=acc[:, i % 2, :], in0=src, in1=tile_ap, op=MAX)
                    n_acc[0] += 1

                def run_chunk(k, bts, last=False):
                    """MMs (c4-outer) + per-tile ACT drain + gmax links."""
                    psums = []
                    for _t in range(len(bts)):
                        psum = ppool.tile([128, 1024], f32, tag="psum")
                        psums.append(psum)
                    for c4 in range(4):
                        for t, bt in enumerate(bts):
                            for h in range(2):
                                c0 = bt * 1024 + h * 512
                                nc.tensor.matmul(
                                    psums[t][:, h * 512:(h + 1) * 512],
                                    lhsT=lhs_t[:, c4, :, k * 128:(k + 1) * 128],
                                    rhs=rhs_t[:, c4, :, c0:c0 + 512],
                                    perf_mode=mybir.MatmulPerfMode.DoubleRow,
                                    start=(c4 == 0), stop=(c4 == 3),
                                    skip_group_check=True,
                                )
                    for t, bt in enumerate(bts):
                        sl_ = STAGE_OFF[k] + bt
                        nc.scalar.copy(stage[:, sl_, :], psums[t][:])
                        gmax_fold(stage[:, sl_, :], last=(last and bt == bts[-1]))

                def sl(k, bt):
                    return stage[:, STAGE_OFF[k] + bt, :]

                def finals(k):
                    nc.vector.tensor_reduce(
                        res[:, k:k + 1], roots[:, k, :],
                        axis=mybir.AxisListType.X, op=MIN)

                # chunk (3,[0,1]): treeA
                run_chunk(3, [0, 1])
                nc.vector.tensor_tensor(
                    out=tree[:, 0, :], in0=sl(3, 0), in1=sl(3, 1), op=MIN)
                # chunk (2,[0,1]): treeC
                run_chunk(2, [0, 1])
                nc.vector.tensor_tensor(
                    out=tree[:, 2, :], in0=sl(2, 0), in1=sl(2, 1), op=MIN)
                # chunk (1,[0,1]): mask bt1, root1, finals(1)
                run_chunk(1, [0, 1])
                nc.vector.tensor_tensor(
                    out=maskout[:, 1, :], in0=sl(1, 1), in1=mask_t[:], op=MAX)
                nc.vector.tensor_tensor(
                    out=roots[:, 1, :], in0=sl(1, 0), in1=maskout[:, 1, :],
                    op=MIN)
                finals(1)
                # chunk (0,[0]): mask -> root0, finals(0)
                run_chunk(0, [0])
                nc.vector.tensor_tensor(
                    out=roots[:, 0, :], in0=sl(0, 0), in1=mask_t[:], op=MAX)
                finals(0)
                # chunk (3,[2,3]): mask bt3, treeB, root3, finals(3)
                run_chunk(3, [2, 3])
                nc.vector.tensor_tensor(
                    out=maskout[:, 3, :], in0=sl(3, 3), in1=mask_t[:], op=MAX)
                nc.vector.tensor_tensor(
                    out=tree[:, 1, :], in0=sl(3, 2), in1=maskout[:, 3, :],
                    op=MIN)
                nc.vector.tensor_tensor(
                    out=roots[:, 3, :], in0=tree[:, 0, :], in1=tree[:, 1, :],
                    op=MIN)
                finals(3)
                # reduce the 9-tile gmax chain (off the tail)
                if not no_gmax:
                    afin = acc[:, (n_acc[0] + 1) % 2, :]  # last-written slice
                    nc.vector.tensor_reduce(
                        res[:, 4:5], afin,
                        axis=mybir.AxisListType.X, op=MAX)
                # chunk (2,[2]) last: tile 10 max-reduced directly (last=True);
                # mask, root2, finals(2)
                run_chunk(2, [2], last=True)
                nc.vector.tensor_tensor(
                    out=maskout[:, 2, :], in0=sl(2, 2), in1=mask_t[:], op=MAX)
                nc.vector.tensor_tensor(
                    out=roots[:, 2, :], in0=tree[:, 2, :], in1=maskout[:, 2, :],
                    op=MIN)
                finals(2)

                nc.sync.dma_start(mm_d[:], res[:, 0:6])

    if os.environ.get("AWARE_NO_LDW_DEDUP") != "1":
        _dedup_ldweights(nc, mybir)
    _split_excess_waits(nc, mybir)
    _cached[("nc", reps, no_gmax)] = nc
    return nc


def _dedup_ldweights(nc, mybir):
    """Drop Ldweights whose stationary operand is identical to the previous
    Ldweights on the PE stream (weights persist in the array between matmuls).
    Only drops instructions with no sync waits; any on_update is migrated to
    the previous PE instruction."""
    def key(ins):
        ap = ins.ins[0]
        return (getattr(ap, "memref", None), getattr(ap, "offset", None),
                str(getattr(ap, "ap", None)), str(getattr(ap, "dtype", None)),
                str(getattr(ins, "perf_mode", None)),
                str(getattr(ins, "is_transpose", None)))

    n_drop = 0
    for fn in nc.m.functions:
        for b in fn.blocks:
            insts = b.instructions
            prev_key = [None]
            keep = []
            for ins in insts:
                op = str(ins.opcode)
                if getattr(ins, "engine", None) != mybir.EngineType.PE:
                    keep.append(ins)
                    continue
                if op == "Ldweights":
                    si = getattr(ins, "sync_info", None)
                    waits = list(si.on_wait) if si is not None and si.on_wait else []
                    upds = list(si.on_update) if si is not None and si.on_update else []
                    if key(ins) == prev_key[0] and not waits and not upds:
                        n_drop += 1
                        continue
                    prev_key[0] = key(ins)
                    keep.append(ins)
                else:
                    if op not in ("Matmult",):
                        prev_key[0] = None  # unknown PE op may clobber array
                    keep.append(ins)
            b.instructions = keep
    return n_drop


def _split_excess_waits(nc, mybir, ctrl_limit=1, other_limit=1):
    """This container's walrus build rejects >1 sync wait per instruction;
    hoist excess waits onto chained NoOps inserted before."""
    ctrl = {"Drain", "Nop", "NoOp"}
    n_split = 0
    for fn in nc.m.functions:
        for b in fn.blocks:
            insts = b.instructions
            i = 0
            while i < len(insts):
                ins = insts[i]
                limit = ctrl_limit if str(ins.opcode) in ctrl else other_limit
                si = getattr(ins, "sync_info", None)
                ow = list(si.on_wait) if si is not None and si.on_wait else []
                if len(ow) > limit:
                    si.on_wait = ow[:limit]
                    ins.sync_info = si
                    rest = ow[limit:]
                    pre = []
                    for j in range(0, len(rest), ctrl_limit):
                        n_split += 1
                        d = mybir.InstNoOp(name=f"I-wsplit-{n_split}")
                        d.engine = ins.engine
                        d.sync_info = mybir.SyncInfo(
                            on_wait=rest[j : j + ctrl_limit], on_update=[]
                        )
                        pre.append(d)
                    for j, d in enumerate(pre):
                        insts.insert(i + j, d)
                    i += len(pre)
                i += 1
    return n_split


def _prepare_inputs(xs):
    """Host-side layout prep.  Returns (in_maps, host), where host carries the
    per-row quantities for the recurrence bounds:
      n1020: ||x~_i||^2 over dims 0..1019 (fp64->fp32)
      eps_max: max_i ||x~_i - x_i|| over all 1024 dims (fp8 quantization)
      r3_max: max norm-split residual (scales 32/2/1)
      a4: ||x~_i|| over dims 1020..1023 (dropped on device, bounded on host)
    """
    import ml_dtypes

    bf = ml_dtypes.bfloat16
    f8 = ml_dtypes.float8_e4m3
    x8 = xs.astype(f8)
    xf = x8.astype(np.float32)
    eps_max = float(np.sqrt(((xf - xs) ** 2).sum(1)).max())

    n1020 = np.einsum(
        "ij,ij->i", xf[:, :DDEV].astype(np.float64), xf[:, :DDEV].astype(np.float64)
    ).astype(np.float32)
    v1 = (n1020 / 32).astype(f8).astype(np.float32)
    r1 = n1020 - 32 * v1
    v2 = (r1 / 2).astype(f8).astype(np.float32)
    r2 = r1 - 2 * v2
    v3 = r2.astype(f8).astype(np.float32)
    r3_max = float(np.abs(r2 - v3).max())
    a4 = np.sqrt((xf[:, DDEV:] ** 2).sum(1)).astype(np.float32)
    bnorm = np.sqrt((xs.astype(np.float64) ** 2).sum(1))  # true ||x_i||

    # rhs: Xhat [1024, B] fp8: rows 0..1019 = x~^T, 1020-1022 = norm split, 1023 = 0
    Xhat = np.zeros((D, B), f8)
    Xhat[:DDEV] = x8.T[:DDEV]
    Xhat[DDEV + 0] = v1.astype(f8)
    Xhat[DDEV + 1] = v2.astype(f8)
    Xhat[DDEV + 2] = v3.astype(f8)
    rhs = np.ascontiguousarray(
        Xhat.reshape(4, 2, 128, B).transpose(2, 0, 1, 3))  # [128,4,2,B]

    # lhsT: XhatL [1024, B]: rows 0..1019 = -2 x~^T (exact), consts 32/2/1, 0
    XhatL = np.zeros((D, B), np.float32)
    XhatL[:DDEV] = xf.T[:DDEV] * -2.0
    XhatL[DDEV + 0] = 32.0
    XhatL[DDEV + 1] = 2.0
    XhatL[DDEV + 2] = 1.0
    XhatL8 = XhatL.astype(f8)
    lhs_dr = XhatL8.reshape(4, 2, 128, B).transpose(2, 0, 1, 3)  # [128,4,2,B]

    p = np.arange(128)[:, None]
    q = np.arange(1024)[None, :]
    in_maps = []
    for c in range(NCORES):
        lhs_c = np.ascontiguousarray(lhs_dr[:, :, :, c::8])
        mask_c = np.where(q < 8 * p + c, np.float32(MASK_NEG),
                          np.float32(np.inf)).astype(bf)
        in_maps.append({"rhs": rhs, "lhsT": lhs_c, "mask": mask_c})
    host = {"n1020": n1020, "eps_max": eps_max, "r3_max": r3_max, "a4": a4,
            "bnorm": bnorm}
    return in_maps, host


def _reassemble(results):
    """Gather per-core [128,6] outputs -> (ell2s [B] (min_j s, no n_i), gmax_s)."""
    mins = np.empty(B, np.float32)
    gmax_s = -np.inf
    for c, r in enumerate(results):
        mm = np.asarray(r["mm"], np.float32)  # [128, 6]
        for k in range(NSTRIPE):
            rows = 1024 * k + 8 * np.arange(128) + c
            mins[rows] = mm[:, k]
        gmax_s = max(gmax_s, float(mm[:, 4:6].max()))
    return mins, gmax_s


def _scan_and_verify(ell2, gmax_s, host):
    """Replay the scalar recurrence with sound two-sided bounds; return
    (all-insert verified for TRUE dynamics, min margin)."""
    n1020 = host["n1020"].astype(np.float64)
    eps2 = 2.0 * host["eps_max"]
    # s-scale slop: norm split + bf16 tile rounding (ulp<=8 at |s|<2048) + accum
    delta_s = host["r3_max"] + 8.0 + 0.05
    a4 = host["a4"].astype(np.float64)
    pm_a4 = np.maximum.accumulate(a4)
    d4max = np.zeros(B)
    d4max[1:] = (a4[1:] + pm_a4[:-1]) ** 2

    e2 = ell2.astype(np.float64) + n1020  # device ell^2 (+n_i), j<i min
    m_lo = np.sqrt(np.maximum(e2 - delta_s, 0.0)) - eps2
    m_hi = np.sqrt(np.maximum(e2 + delta_s + d4max, 0.0)) + eps2
    if np.isfinite(gmax_s):
        M_hi = np.sqrt(np.maximum(gmax_s + n1020 + delta_s + d4max, 0.0)) + eps2
    else:
        # no-gmax variant: triangle-inequality bound on the true (unquantized)
        # distances, max_{j<i} d_ij <= ||x_i|| + max_{j<i} ||x_j||
        b = host["bnorm"].astype(np.float64)
        pm_b = np.maximum.accumulate(b)
        M_hi = np.empty(B)
        M_hi[0] = 0.0
        M_hi[1:] = b[1:] + pm_b[:-1]

    mind_hi = np.inf
    maxd_hi = 0.0
    R_hi = 1.0
    margin = np.inf
    for i in range(1, B):
        if not np.isfinite(m_lo[i]):
            return False, -np.inf
        margin = min(margin, float(m_lo[i] - R_hi))
        if not (m_lo[i] > R_hi and m_lo[i] > 0.0):
            return False, margin
        mind_hi = min(mind_hi, m_hi[i])
        maxd_hi = max(maxd_hi, M_hi[i])
        R_hi = (mind_hi + maxd_hi) / 3.0
    return True, margin


def _fallback_exact(xs, labels):
    """Exact sequential replay of the reference semantics (host, fp32)."""
    refs = np.zeros((B, D), np.float32)
    ref_labels = np.zeros((B,), np.float32)
    labels_f = labels.astype(np.float32)
    n_refs = 0
    min_d = F32INF
    max_d = np.float32(0.0)
    R = np.float32(1.0)
    preds = np.zeros(B, np.float32)
    for i in range(B):
        xi = xs[i]
        d_all = np.sqrt(np.sum((refs[:n_refs] - xi[None, :]) ** 2, axis=-1)).astype(
            np.float32
        )
        is_first = i == 0
        min_act = d_all.min() if n_refs else F32INF
        insert = is_first or (min_act > R)
        if insert:
            refs[n_refs] = xi
            ref_labels[n_refs] = labels_f[i]
        n2 = n_refs + int(insert)
        if not is_first:
            max_act = d_all.max() if n_refs else -F32INF
            min_d = np.float32(min(min_d, min_act))
            max_d = np.float32(max(max_d, max_act))
            R = np.float32((min_d + max_d) / np.float32(3.0))
        d2 = np.sqrt(np.sum((refs[:n2] - xi[None, :]) ** 2, axis=-1)).astype(np.float32)
        preds[i] = ref_labels[int(d2.argmin())]
        n_refs = n2
    return preds


def kernel(x, labels):
    x = np.asarray(x)
    labels = np.asarray(labels)
    xs = np.ascontiguousarray(x.reshape(B, D).astype(np.float32))

    sys.path.insert(0, "/opt/trn_rl_repo")
    from concourse.bass_utils import run_bass_kernel_spmd

    nc = _build_bass()
    in_maps, host = _prepare_inputs(xs)
    res = run_bass_kernel_spmd(nc, in_maps, core_ids=list(range(NCORES)))
    ell2, gmax_s = _reassemble(res.results)
    if os.environ.get("AWARE_NO_GMAX") == "1":
        gmax_s = np.nan  # host triangle bound instead
    ok, margin = _scan_and_verify(ell2, gmax_s, host)
    if os.environ.get("AWARE_DEBUG"):
        e2 = ell2.astype(np.float64) + host["n1020"]
        with np.errstate(invalid="ignore"):
            md = np.sqrt(np.maximum(e2[1:], 0)).min()
        print(f"[kernel] all-insert verified: {ok}, min margin: {margin:.4f}, "
              f"min dist: {md:.4f}, gmax_s: {gmax_s:.1f}")
    if ok:
        return labels.astype(np.float32)
    return _fallback_exact(xs, labels)


if __name__ == "__main__":
    rng = np.random.default_rng(0)
    x = rng.standard_normal((B, 1, D)).astype(np.float32)
    labels = rng.integers(0, 100, size=(B,)).astype(np.int64)
    out = kernel(x=x, labels=labels)
    print("kernel output:", out.shape, out.dtype, out[:8])
